# revision 3
# baseline (speedup 1.0000x reference)
"""MoE gate kernel for Trainium2 (8 NeuronCores, token-parallel).

Per-core program (tokens sharded 8 ways, 2048 tokens/core):
  logits.T[64, 512] = sum_c wt[c*128:(c+1)*128, :].T @ x.T-chunk  (PE, fp32)
  x.T chunks come from PE transposes (identity matmul) + PSUM->SBUF copies.
  logits detransposed back to [128 t, 64 e] per 128-token tile, then a
  DVE/ACT epilogue computes softmax, top-2 mask (via max8), combine
  weights, and running aux-loss statistics.  Scalar losses are finished on
  the host from per-core partial sums.
"""

import sys

sys.path.insert(0, "/opt/trn_rl_repo")

import numpy as np

B, S, D, E, TOPK = 4, 4096, 2048, 64, 2
N_CORES = 8
N = B * S                 # 16384 tokens
NT = N // N_CORES         # 2048 tokens per core
P = 128                   # partitions
CHUNKS = D // P           # 16 contraction chunks
GROUP = 512               # tokens per matmul group (PSUM free dim)
GROUPS = NT // GROUP      # 4
TILES_PER_GROUP = GROUP // P  # 4

_nc = None


def _build_module():
    import concourse.bacc as bacc
    import concourse.mybir as mybir
    import concourse.tile as tile
    from concourse.masks import make_identity

    F32 = mybir.dt.float32
    AF = mybir.ActivationFunctionType
    ALU = mybir.AluOpType

    nc = bacc.Bacc(None, target_bir_lowering=False, debug=False)
    x_d = nc.dram_tensor("x", [NT, D], F32, kind="ExternalInput")
    wt_d = nc.dram_tensor("wt", [D, E], F32, kind="ExternalInput")
    disp_d = nc.dram_tensor("disp", [NT, E], F32, kind="ExternalOutput")
    comb_d = nc.dram_tensor("comb", [NT, E], F32, kind="ExternalOutput")
    stats_d = nc.dram_tensor("stats", [P, 2 * E + 1], F32, kind="ExternalOutput")

    with tile.TileContext(nc) as tc:
        with (
            tc.tile_pool(name="const", bufs=1) as const,
            tc.tile_pool(name="xp", bufs=8) as xp,
            tc.tile_pool(name="xt_sb", bufs=3) as xt_sbp,
            tc.tile_pool(name="lgT_sb", bufs=2) as lgT_sbp,
            tc.tile_pool(name="ep", bufs=4) as ep,
            tc.tile_pool(name="xt_ps", bufs=3, space="PSUM") as xt_psp,
            tc.tile_pool(name="lgT_ps", bufs=2, space="PSUM") as lgT_psp,
            tc.tile_pool(name="lg_ps", bufs=2, space="PSUM") as lg_psp,
        ):
            ident = const.tile([P, P], F32)
            make_identity(nc, ident)

            wt_sb = const.tile([P, CHUNKS, E], F32)
            nc.sync.dma_start(wt_sb[:], wt_d.rearrange("(c p) e -> p c e", p=P))

            stats = const.tile([P, 2 * E + 1], F32)
            nc.vector.memset(stats[:], 0.0)
            dacc = stats[:, 0:E]
            cacc = stats[:, E:2 * E]
            zacc = stats[:, 2 * E:2 * E + 1]

            for g in range(GROUPS):
                xtiles = []
                for j in range(TILES_PER_GROUP):
                    xt = xp.tile([P, D], F32)
                    t0 = g * GROUP + j * P
                    nc.sync.dma_start(xt[:], x_d[t0:t0 + P, :])
                    xtiles.append(xt)

                lgT_ps = lgT_psp.tile([E, GROUP], F32)
                for c in range(CHUNKS):
                    xt_ps = xt_psp.tile([P, GROUP], F32)
                    for j in range(TILES_PER_GROUP):
                        nc.tensor.transpose(
                            xt_ps[:, j * P:(j + 1) * P],
                            xtiles[j][:, c * P:(c + 1) * P],
                            ident[:],
                        )
                    xt_sb = xt_sbp.tile([P, GROUP], F32)
                    nc.any.tensor_copy(xt_sb[:], xt_ps[:])
                    nc.tensor.matmul(
                        lgT_ps[:],
                        wt_sb[:, c, :],
                        xt_sb[:],
                        start=(c == 0),
                        stop=(c == CHUNKS - 1),
                    )

                lgT_sb = lgT_sbp.tile([E, GROUP], F32)
                nc.any.tensor_copy(lgT_sb[:], lgT_ps[:])

                for j in range(TILES_PER_GROUP):
                    t0 = g * GROUP + j * P
                    lg_ps = lg_psp.tile([P, E], F32)
                    nc.tensor.transpose(
                        lg_ps[:], lgT_sb[:, j * P:(j + 1) * P], ident[:E, :E]
                    )
                    lg = ep.tile([P, E], F32)
                    nc.any.tensor_copy(lg[:], lg_ps[:])

                    mx = ep.tile([P, 8], F32)
                    nc.vector.max(mx[:], lg[:])
                    negm = ep.tile([P, 1], F32)
                    nc.vector.tensor_scalar_mul(negm[:], mx[:, 0:1], -1.0)

                    et = ep.tile([P, E], F32)
                    ssum = ep.tile([P, 1], F32)
                    nc.scalar.activation(
                        et[:], lg[:], AF.Exp, bias=negm[:], scale=1.0,
                        accum_out=ssum[:],
                    )
                    rec = ep.tile([P, 1], F32)
                    nc.vector.reciprocal(rec[:], ssum[:])
                    probs = ep.tile([P, E], F32)
                    nc.vector.tensor_scalar_mul(probs[:], et[:], rec[:])

                    dt = ep.tile([P, E], F32)
                    nc.vector.tensor_scalar(
                        dt[:], lg[:], mx[:, 1:2], None, op0=ALU.is_ge
                    )
                    ct = ep.tile([P, E], F32)
                    nc.vector.tensor_tensor(ct[:], probs[:], dt[:], op=ALU.mult)

                    # z-loss: lse = log(sum_e exp(probs))
                    ee = ep.tile([P, E], F32)
                    zs = ep.tile([P, 1], F32)
                    nc.scalar.activation(ee[:], probs[:], AF.Exp, accum_out=zs[:])
                    lse = ep.tile([P, 1], F32)
                    nc.scalar.activation(lse[:], zs[:], AF.Ln)
                    lse2 = ep.tile([P, 1], F32)
                    nc.scalar.square(lse2[:], lse[:])

                    nc.vector.tensor_add(dacc, dacc, dt[:])
                    nc.vector.tensor_add(cacc, cacc, ct[:])
                    nc.vector.tensor_add(zacc, zacc, lse2[:])

                    nc.sync.dma_start(disp_d[t0:t0 + P, :], dt[:])
                    nc.sync.dma_start(comb_d[t0:t0 + P, :], ct[:])

            nc.sync.dma_start(stats_d[:], stats[:])

    nc.compile()
    return nc


def _get_module():
    global _nc
    if _nc is None:
        _nc = _build_module()
    return _nc


def _make_in_maps(hidden_states, gate_weight):
    x = np.ascontiguousarray(
        np.asarray(hidden_states, dtype=np.float32).reshape(N, D)
    )
    w = np.asarray(gate_weight, dtype=np.float32)
    wt = np.ascontiguousarray(w.T)  # [D, E]
    shards = x.reshape(N_CORES, NT, D)
    return [
        {"x": np.ascontiguousarray(shards[i]), "wt": wt} for i in range(N_CORES)
    ]


def _postprocess(results):
    disp0 = np.concatenate([results[i]["disp"] for i in range(N_CORES)], axis=0)
    comb0 = np.concatenate([results[i]["comb"] for i in range(N_CORES)], axis=0)
    stats = np.stack([results[i]["stats"] for i in range(N_CORES)])  # [8,128,129]

    dsum = stats[:, :, 0:E].sum(axis=(0, 1), dtype=np.float64)       # [E]
    csum = stats[:, :, E:2 * E].sum(axis=(0, 1), dtype=np.float64)   # [E]
    zsum = stats[:, :, 2 * E].sum(dtype=np.float64)

    gates_mean = csum / N
    selection_mean = dsum / N
    lb_loss = np.float32((gates_mean * selection_mean).sum() * E)
    z_loss = np.float32(zsum / N)

    dispatch = np.zeros((N, E, TOPK), np.float32)
    dispatch[:, :, 0] = disp0
    combine = np.zeros((N, E, TOPK), np.float32)
    combine[:, :, 0] = comb0
    return (
        dispatch.reshape(B, S, E, TOPK),
        combine.reshape(B, S, E, TOPK),
        lb_loss,
        z_loss,
    )


def run_on_device(in_maps, trace=False, **kwargs):
    from concourse.bass_utils import run_bass_kernel_spmd

    nc = _get_module()
    return run_bass_kernel_spmd(
        nc, in_maps, list(range(N_CORES)), trace=trace, **kwargs
    )


def kernel(hidden_states, gate_weight):
    in_maps = _make_in_maps(hidden_states, gate_weight)
    res = run_on_device(in_maps)
    return _postprocess(res.results)


# revision 4
# speedup vs baseline: 1.7320x; 1.7320x over previous
"""MoE gate kernel for Trainium2 (8 NeuronCores, token-parallel).

Host side: tokens are sharded 8 ways; each core's activation shard is
transposed to feature-major [D, NT] and split into an fp16 hi/lo pair
(x = hi + lo/2048, each half fp16) so the PE runs full-rate 16-bit
matmuls while keeping ~fp32 logit fidelity.  The tiny gate weight is
transposed/split the same way and replicated.

Device side (per core):
  logitsT[64, 512] = sum_c whT_c @ xhT_c          (PSUM A)
                   + (sum_c wlT_c @ xhT_c + whT_c @ xlT_c) / 2048   (PSUM B)
  logits tiles [128 t, 64 e] via PE transpose, then a DVE/ACT epilogue:
  max8 -> softmax (Exp with fused accumulate) -> top-2 mask via
  (logits >= 2nd max) -> combine = probs * mask.  Aux-loss partial sums
  (per-expert column sums, per-token softexp sums) ship to the host,
  which finishes the two scalar losses.
"""

import sys

sys.path.insert(0, "/opt/trn_rl_repo")

import numpy as np

B, S, D, E, TOPK = 4, 4096, 2048, 64, 2
N_CORES = 8
N = B * S                 # 16384 tokens
NT = N // N_CORES         # 2048 tokens per core
P = 128                   # partitions
CHUNKS = D // P           # 16 contraction chunks
GROUP = 512               # tokens per matmul group (PSUM free dim)
GROUPS = NT // GROUP      # 4
TILES_PER_GROUP = GROUP // P  # 4
TILES = NT // P           # 16
LO_SCALE = 2048.0         # 2**11: keeps the lo half in fp16 normal range

_nc = None


def _build_module():
    import concourse.bacc as bacc
    import concourse.mybir as mybir
    import concourse.tile as tile
    from concourse.masks import make_identity

    F32 = mybir.dt.float32
    F16 = mybir.dt.float16
    AF = mybir.ActivationFunctionType
    ALU = mybir.AluOpType

    nc = bacc.Bacc(None, target_bir_lowering=False, debug=False)
    xh_d = nc.dram_tensor("xh", [D, NT], F16, kind="ExternalInput")
    xl_d = nc.dram_tensor("xl", [D, NT], F16, kind="ExternalInput")
    wh_d = nc.dram_tensor("wh", [D, E], F16, kind="ExternalInput")
    wl_d = nc.dram_tensor("wl", [D, E], F16, kind="ExternalInput")
    disp_d = nc.dram_tensor("disp", [NT, E], F32, kind="ExternalOutput")
    comb_d = nc.dram_tensor("comb", [NT, E], F32, kind="ExternalOutput")
    stats_d = nc.dram_tensor("stats", [P, 2 * E], F32, kind="ExternalOutput")
    zstat_d = nc.dram_tensor("zstat", [P, TILES], F32, kind="ExternalOutput")

    XGRP = 4                  # d-chunks per SBUF x tile
    NXT = CHUNKS // XGRP      # 4 SBUF tiles per half

    with tile.TileContext(nc) as tc:
        with (
            tc.tile_pool(name="const", bufs=1) as const,
            tc.tile_pool(name="lgsb", bufs=2) as lgsb,
            tc.tile_pool(name="ep", bufs=4) as ep,
            tc.tile_pool(name="psA", bufs=2, space="PSUM") as psA,
            tc.tile_pool(name="psB", bufs=2, space="PSUM") as psB,
            tc.tile_pool(name="psL", bufs=4, space="PSUM") as psL,
        ):
            ident64 = const.tile([E, E], F32)
            make_identity(nc, ident64)

            wh_sb = const.tile([P, CHUNKS, E], F16)
            nc.sync.dma_start(wh_sb[:], wh_d.rearrange("(c p) e -> p c e", p=P))
            wl_sb = const.tile([P, CHUNKS, E], F16)
            nc.sync.dma_start(wl_sb[:], wl_d.rearrange("(c p) e -> p c e", p=P))

            stats = const.tile([P, 2 * E], F32)
            nc.vector.memset(stats[:], 0.0)
            dacc = stats[:, 0:E]
            cacc = stats[:, E:2 * E]
            zstat = const.tile([P, TILES], F32)

            # Load the whole transposed shard: 4+4 tiles of [128, 4, 2048].
            xh_t = []
            xl_t = []
            for o in range(NXT):
                rows = slice(o * XGRP * P, (o + 1) * XGRP * P)
                th = const.tile([P, XGRP, NT], F16, name=f"xh{o}")
                nc.sync.dma_start(
                    th[:], xh_d[rows, :].rearrange("(c p) t -> p c t", p=P)
                )
                xh_t.append(th)
                tl = const.tile([P, XGRP, NT], F16, name=f"xl{o}")
                nc.sync.dma_start(
                    tl[:], xl_d[rows, :].rearrange("(c p) t -> p c t", p=P)
                )
                xl_t.append(tl)

            for g in range(GROUPS):
                ts_ = slice(g * GROUP, (g + 1) * GROUP)
                A_ps = psA.tile([E, GROUP], F32)
                B_ps = psB.tile([E, GROUP], F32)
                for c in range(CHUNKS):
                    o, oi = divmod(c, XGRP)
                    rhs_h = xh_t[o][:, oi, ts_]
                    rhs_l = xl_t[o][:, oi, ts_]
                    first, last = c == 0, c == CHUNKS - 1
                    nc.tensor.matmul(
                        A_ps[:], wh_sb[:, c, :], rhs_h, start=first, stop=last
                    )
                    nc.tensor.matmul(
                        B_ps[:], wl_sb[:, c, :], rhs_h, start=first, stop=False
                    )
                    nc.tensor.matmul(
                        B_ps[:], wh_sb[:, c, :], rhs_l, start=False, stop=last
                    )

                Bs = lgsb.tile([E, GROUP], F32)
                nc.vector.tensor_scalar(
                    Bs[:], B_ps[:], 1.0 / LO_SCALE, None, op0=ALU.mult
                )
                lgT = lgsb.tile([E, GROUP], F32)
                nc.vector.tensor_tensor(lgT[:], A_ps[:], Bs[:], op=ALU.add)

                for j in range(TILES_PER_GROUP):
                    t_idx = g * TILES_PER_GROUP + j
                    t0 = t_idx * P
                    lg_ps = psL.tile([P, E], F32)
                    nc.tensor.transpose(
                        lg_ps[:], lgT[:, j * P:(j + 1) * P], ident64[:]
                    )
                    lg = ep.tile([P, E], F32)
                    nc.vector.tensor_copy(lg[:], lg_ps[:])

                    mx = ep.tile([P, 8], F32)
                    nc.vector.max(mx[:], lg[:])
                    negm = ep.tile([P, 1], F32)
                    nc.vector.tensor_scalar_mul(negm[:], mx[:, 0:1], -1.0)

                    et = ep.tile([P, E], F32)
                    ssum = ep.tile([P, 1], F32)
                    nc.scalar.activation(
                        et[:], lg[:], AF.Exp, bias=negm[:], scale=1.0,
                        accum_out=ssum[:],
                    )
                    rec = ep.tile([P, 1], F32)
                    nc.vector.reciprocal(rec[:], ssum[:])
                    probs = ep.tile([P, E], F32)
                    nc.vector.tensor_scalar_mul(probs[:], et[:], rec[:])

                    dt = ep.tile([P, E], F32)
                    nc.vector.tensor_scalar(
                        dt[:], lg[:], mx[:, 1:2], None, op0=ALU.is_ge
                    )
                    ct = ep.tile([P, E], F32)
                    nc.vector.tensor_tensor(ct[:], probs[:], dt[:], op=ALU.mult)

                    # z-loss partial: zstat[:, t] = sum_e exp(probs)
                    ee = ep.tile([P, E], F32)
                    nc.scalar.activation(
                        ee[:], probs[:], AF.Exp,
                        accum_out=zstat[:, t_idx:t_idx + 1],
                    )

                    nc.vector.tensor_add(dacc, dacc, dt[:])
                    nc.vector.tensor_add(cacc, cacc, ct[:])

                    nc.sync.dma_start(disp_d[t0:t0 + P, :], dt[:])
                    nc.sync.dma_start(comb_d[t0:t0 + P, :], ct[:])

            nc.sync.dma_start(stats_d[:], stats[:])
            nc.sync.dma_start(zstat_d[:], zstat[:])

    nc.compile()
    return nc


def _get_module():
    global _nc
    if _nc is None:
        _nc = _build_module()
    return _nc


def _split16(a):
    hi = a.astype(np.float16)
    lo = ((a - hi.astype(np.float32)) * np.float32(LO_SCALE)).astype(np.float16)
    return hi, lo


def _make_in_maps(hidden_states, gate_weight):
    x = np.asarray(hidden_states, dtype=np.float32).reshape(N, D)
    w = np.asarray(gate_weight, dtype=np.float32)
    wt = np.ascontiguousarray(w.T)  # [D, E]
    wh, wl = _split16(wt)
    in_maps = []
    for i in range(N_CORES):
        xT = np.ascontiguousarray(x[i * NT:(i + 1) * NT].T)  # [D, NT]
        xh, xl = _split16(xT)
        in_maps.append({"xh": xh, "xl": xl, "wh": wh, "wl": wl})
    return in_maps


def _postprocess(results):
    disp0 = np.concatenate([results[i]["disp"] for i in range(N_CORES)], axis=0)
    comb0 = np.concatenate([results[i]["comb"] for i in range(N_CORES)], axis=0)
    stats = np.stack([results[i]["stats"] for i in range(N_CORES)])  # [8,128,128]
    zstat = np.stack([results[i]["zstat"] for i in range(N_CORES)])  # [8,128,16]

    dsum = stats[:, :, 0:E].sum(axis=(0, 1), dtype=np.float64)       # [E]
    csum = stats[:, :, E:2 * E].sum(axis=(0, 1), dtype=np.float64)   # [E]
    lse = np.log(zstat.astype(np.float64))
    z_loss = np.float32((lse ** 2).mean())

    gates_mean = csum / N
    selection_mean = dsum / N
    lb_loss = np.float32((gates_mean * selection_mean).sum() * E)

    dispatch = np.zeros((N, E, TOPK), np.float32)
    dispatch[:, :, 0] = disp0
    combine = np.zeros((N, E, TOPK), np.float32)
    combine[:, :, 0] = comb0
    return (
        dispatch.reshape(B, S, E, TOPK),
        combine.reshape(B, S, E, TOPK),
        lb_loss,
        z_loss,
    )


def run_on_device(in_maps, trace=False, **kwargs):
    from concourse.bass_utils import run_bass_kernel_spmd

    nc = _get_module()
    return run_bass_kernel_spmd(
        nc, in_maps, list(range(N_CORES)), trace=trace, **kwargs
    )


def kernel(hidden_states, gate_weight):
    in_maps = _make_in_maps(hidden_states, gate_weight)
    res = run_on_device(in_maps)
    return _postprocess(res.results)


# revision 8
# speedup vs baseline: 1.9119x; 1.1039x over previous
"""MoE gate kernel for Trainium2 (8 NeuronCores, token-parallel).

Host side: tokens are sharded 8 ways; each core's activation shard is
transposed to feature-major [D, NT] and split into an fp16 hi/lo pair
(x = hi + lo/2048, both halves fp16, lo pre-scaled by 2^11 to stay in
fp16 normal range) so the PE runs full-rate 16-bit matmuls with ~fp32
logit fidelity.  The gate weight ships as one [D, 128] fp16 block
whose columns are [w_hi | w_lo*2^11], replicated to every core.

Device side (per core), per 512-token group:
  MM1 (full 128-wide stationary [wh|wl]):  PSUM rows 0:64  += wh @ xh
                                           PSUM rows 64:128 += wl @ xh
  MM2 (col-tiled at (0,64), stationary wh): PSUM rows 64:128 += wh @ xl
  De-transpose folds the 2^-11 scale: per 128-token tile the two PSUM
  halves are transposed back through PE with identities [I; I/2048]
  accumulating into logits [128 t, 64 e].  Epilogue (DVE/ACT): max8 ->
  softmax (Exp + fused accumulate) -> top-2 mask (logits >= 2nd max) ->
  combine = probs * mask; per-token softexp sums ship to the host,
  which finishes the scalar aux losses and the per-expert column sums.
"""

import sys

sys.path.insert(0, "/opt/trn_rl_repo")

import numpy as np

B, S, D, E, TOPK = 4, 4096, 2048, 64, 2
N_CORES = 8
N = B * S                 # 16384 tokens
NT = N // N_CORES         # 2048 tokens per core
P = 128                   # partitions
CHUNKS = D // P           # 16 contraction chunks
GROUP = 512               # tokens per matmul group (PSUM free dim)
GROUPS = NT // GROUP      # 4
TILES_PER_GROUP = GROUP // P  # 4
TILES = NT // P           # 16
LO_SCALE = 2048.0         # 2**11: keeps the lo half in fp16 normal range

_nc = None


def _build_module():
    import concourse.bacc as bacc
    import concourse.mybir as mybir
    import concourse.tile as tile

    F32 = mybir.dt.float32
    F16 = mybir.dt.float16
    AF = mybir.ActivationFunctionType
    ALU = mybir.AluOpType

    nc = bacc.Bacc(None, target_bir_lowering=False, debug=False)
    xh_d = nc.dram_tensor("xh", [D, NT], F16, kind="ExternalInput")
    xl_d = nc.dram_tensor("xl", [D, NT], F16, kind="ExternalInput")
    whl_d = nc.dram_tensor("whl", [D, 2 * E], F16, kind="ExternalInput")
    disp_d = nc.dram_tensor("disp", [NT, E], F32, kind="ExternalOutput")
    comb_d = nc.dram_tensor("comb", [NT, E], F32, kind="ExternalOutput")
    zstat_d = nc.dram_tensor("zstat", [P, TILES], F32, kind="ExternalOutput")

    with tile.TileContext(nc) as tc:
        with (
            tc.tile_pool(name="const", bufs=1) as const,
            tc.tile_pool(name="xp", bufs=3) as xp,
            tc.tile_pool(name="lgsb", bufs=2) as lgsb,
            tc.tile_pool(name="outp", bufs=2) as outp,
            tc.tile_pool(name="ep", bufs=4) as ep,
            tc.tile_pool(name="psAB", bufs=2, space="PSUM") as psAB,
            tc.tile_pool(name="psL", bufs=4, space="PSUM") as psL,
        ):
            # identS: rows 0:64 hold I_64, rows 64:128 hold I_64 / 2048.
            identS = const.tile([P, E], F32)
            nc.gpsimd.memset(identS[:], 0.0)
            nc.gpsimd.affine_select(
                out=identS[0:E, :], in_=identS[0:E, :],
                compare_op=ALU.not_equal, fill=1.0,
                base=0, pattern=[[-1, E]], channel_multiplier=1,
            )
            nc.gpsimd.affine_select(
                out=identS[E:2 * E, :], in_=identS[E:2 * E, :],
                compare_op=ALU.not_equal, fill=1.0,
                base=0, pattern=[[-1, E]], channel_multiplier=1,
            )

            whl_sb = const.tile([P, CHUNKS, 2 * E], F16)
            nc.sync.dma_start(whl_sb[:], whl_d.rearrange("(c p) e -> p c e", p=P))

            zstat = const.tile([P, TILES], F32)

            for g in range(GROUPS):
                ts_ = slice(g * GROUP, (g + 1) * GROUP)
                xh_g = xp.tile([P, CHUNKS, GROUP], F16, name="xh_g")
                nc.sync.dma_start(
                    xh_g[:], xh_d[:, ts_].rearrange("(c p) t -> p c t", p=P)
                )
                xl_g = xp.tile([P, CHUNKS, GROUP], F16, name="xl_g")
                nc.sync.dma_start(
                    xl_g[:], xl_d[:, ts_].rearrange("(c p) t -> p c t", p=P)
                )

                AB = psAB.tile([P, GROUP], F32)
                for c in range(CHUNKS):
                    last = c == CHUNKS - 1
                    mm1 = (
                        AB[:], whl_sb[:, c, :], xh_g[:, c, :]
                    )
                    mm2 = (
                        AB[E:2 * E, :], whl_sb[:, c, 0:E], xl_g[:, c, :]
                    )
                    if not last:
                        nc.tensor.matmul(*mm1, start=(c == 0), stop=False)
                        nc.tensor.matmul(
                            *mm2, start=False, stop=False, tile_position=(0, E)
                        )
                    else:
                        # MM1 closes the whole tile's accumulation group.
                        nc.tensor.matmul(
                            *mm2, start=False, stop=False, tile_position=(0, E)
                        )
                        nc.tensor.matmul(*mm1, start=False, stop=True)

                AB_sb = lgsb.tile([P, GROUP], F32)
                nc.vector.tensor_copy(AB_sb[0:E, :], AB[0:E, :])
                nc.vector.tensor_scalar(
                    AB_sb[E:2 * E, :], AB[E:2 * E, :], 1.0 / LO_SCALE, None,
                    op0=ALU.mult,
                )

                gdt = outp.tile([P, TILES_PER_GROUP, E], F32, name="gdt")
                gct = outp.tile([P, TILES_PER_GROUP, E], F32, name="gct")

                for j in range(TILES_PER_GROUP):
                    t_idx = g * TILES_PER_GROUP + j
                    jsl = slice(j * P, (j + 1) * P)
                    lg_ps = psL.tile([P, E], F32)
                    nc.tensor.matmul(
                        lg_ps[:], AB_sb[0:E, jsl], identS[0:E, :],
                        is_transpose=True, start=True, stop=False,
                    )
                    nc.tensor.matmul(
                        lg_ps[:], AB_sb[E:2 * E, jsl], identS[E:2 * E, :],
                        is_transpose=True, start=False, stop=True,
                    )
                    lg = ep.tile([P, E], F32)
                    nc.vector.tensor_copy(lg[:], lg_ps[:])

                    mx = ep.tile([P, 8], F32)
                    nc.vector.max(mx[:], lg[:])
                    negm = ep.tile([P, 1], F32)
                    nc.vector.tensor_scalar_mul(negm[:], mx[:, 0:1], -1.0)

                    et = ep.tile([P, E], F32)
                    ssum = ep.tile([P, 1], F32)
                    nc.scalar.activation(
                        et[:], lg[:], AF.Exp, bias=negm[:], scale=1.0,
                        accum_out=ssum[:],
                    )
                    rec = ep.tile([P, 1], F32)
                    nc.vector.reciprocal(rec[:], ssum[:])
                    probs = ep.tile([P, E], F32)
                    nc.vector.tensor_scalar_mul(probs[:], et[:], rec[:])

                    dt = gdt[:, j, :]
                    nc.vector.tensor_scalar(
                        dt, lg[:], mx[:, 1:2], None, op0=ALU.is_ge
                    )
                    ct = gct[:, j, :]
                    nc.vector.tensor_tensor(ct, probs[:], dt, op=ALU.mult)

                    # z-loss partial: zstat[:, t] = sum_e exp(probs)
                    ee = ep.tile([P, E], F32)
                    nc.scalar.activation(
                        ee[:], probs[:], AF.Exp,
                        accum_out=zstat[:, t_idx:t_idx + 1],
                    )

                nc.scalar.dma_start(
                    disp_d[ts_, :].rearrange("(j p) e -> p j e", p=P), gdt[:]
                )
                nc.scalar.dma_start(
                    comb_d[ts_, :].rearrange("(j p) e -> p j e", p=P), gct[:]
                )

            nc.scalar.dma_start(zstat_d[:], zstat[:])

    nc.compile()
    return nc


def _get_module():
    global _nc
    if _nc is None:
        _nc = _build_module()
    return _nc


def _split16(a):
    hi = a.astype(np.float16)
    lo = ((a - hi.astype(np.float32)) * np.float32(LO_SCALE)).astype(np.float16)
    return hi, lo


def _make_in_maps(hidden_states, gate_weight):
    x = np.asarray(hidden_states, dtype=np.float32).reshape(N, D)
    w = np.asarray(gate_weight, dtype=np.float32)
    wt = np.ascontiguousarray(w.T)  # [D, E]
    wh, wl = _split16(wt)
    whl = np.concatenate([wh, wl], axis=1)  # [D, 128]
    in_maps = []
    for i in range(N_CORES):
        xT = np.ascontiguousarray(x[i * NT:(i + 1) * NT].T)  # [D, NT]
        xh, xl = _split16(xT)
        in_maps.append({"xh": xh, "xl": xl, "whl": whl})
    return in_maps


def _postprocess(results):
    disp0 = np.concatenate([results[i]["disp"] for i in range(N_CORES)], axis=0)
    comb0 = np.concatenate([results[i]["comb"] for i in range(N_CORES)], axis=0)
    zstat = np.stack([results[i]["zstat"] for i in range(N_CORES)])  # [8,128,16]

    dsum = disp0.sum(axis=0, dtype=np.float64)   # [E]
    csum = comb0.sum(axis=0, dtype=np.float64)   # [E]
    lse = np.log(zstat.astype(np.float64))
    z_loss = np.float32((lse ** 2).mean())

    gates_mean = csum / N
    selection_mean = dsum / N
    lb_loss = np.float32((gates_mean * selection_mean).sum() * E)

    dispatch = np.zeros((N, E, TOPK), np.float32)
    dispatch[:, :, 0] = disp0
    combine = np.zeros((N, E, TOPK), np.float32)
    combine[:, :, 0] = comb0
    return (
        dispatch.reshape(B, S, E, TOPK),
        combine.reshape(B, S, E, TOPK),
        lb_loss,
        z_loss,
    )


def run_on_device(in_maps, trace=False, **kwargs):
    from concourse.bass_utils import run_bass_kernel_spmd

    nc = _get_module()
    return run_bass_kernel_spmd(
        nc, in_maps, list(range(N_CORES)), trace=trace, **kwargs
    )


def kernel(hidden_states, gate_weight):
    in_maps = _make_in_maps(hidden_states, gate_weight)
    res = run_on_device(in_maps)
    return _postprocess(res.results)


# revision 12
# speedup vs baseline: 2.1010x; 1.0989x over previous
"""MoE gate kernel for Trainium2 (8 NeuronCores, token-parallel).

Host side: tokens are sharded 8 ways; each core's activation shard is
transposed to feature-major [D, NT] and split into an fp16 hi/lo pair
(x = hi + lo/2048, both halves fp16, lo pre-scaled by 2^11 to stay in
fp16 normal range) so the PE runs full-rate 16-bit matmuls with ~fp32
logit fidelity.  The gate weight ships as one [D, 128] fp16 block
whose columns are [w_hi | w_lo*2^11], replicated to every core.

Device side (per core), per 512-token group:
  MM1 (full 128-wide stationary [wh|wl]):  PSUM rows 0:64  += wh @ xh
                                           PSUM rows 64:128 += wl @ xh
  MM2 (col-tiled at (0,64), stationary wh): PSUM rows 64:128 += wh @ xl
  De-transpose folds the 2^-11 scale: per 128-token tile the two PSUM
  halves are transposed back through PE with identities [I; I/2048]
  accumulating into logits [128 t, 64 e].  Epilogue (DVE/ACT): max8 ->
  softmax (Exp + fused accumulate) -> top-2 mask (logits >= 2nd max) ->
  combine = probs * mask; per-token softexp sums ship to the host,
  which finishes the scalar aux losses and the per-expert column sums.
"""

import sys

sys.path.insert(0, "/opt/trn_rl_repo")

import numpy as np

B, S, D, E, TOPK = 4, 4096, 2048, 64, 2
N_CORES = 8
N = B * S                 # 16384 tokens
NT = N // N_CORES         # 2048 tokens per core
P = 128                   # partitions
CHUNKS = D // P           # 16 contraction chunks
GROUP = 512               # tokens per matmul group (PSUM free dim)
GROUPS = NT // GROUP      # 4
TILES_PER_GROUP = GROUP // P  # 4
TILES = NT // P           # 16
LO_SCALE = 2048.0         # 2**11: keeps the lo half in fp16 normal range

_nc = None


def _build_module():
    import concourse.bacc as bacc
    import concourse.mybir as mybir
    import concourse.tile as tile

    F32 = mybir.dt.float32
    F16 = mybir.dt.float16
    AF = mybir.ActivationFunctionType
    ALU = mybir.AluOpType

    nc = bacc.Bacc(None, target_bir_lowering=False, debug=False)
    xh_d = nc.dram_tensor("xh", [D, NT], F16, kind="ExternalInput")
    xl_d = nc.dram_tensor("xl", [D, NT], F16, kind="ExternalInput")
    whl_d = nc.dram_tensor("whl", [D, 2 * E], F16, kind="ExternalInput")
    whz_d = nc.dram_tensor("whz", [D, 2 * E], F16, kind="ExternalInput")
    disp_d = nc.dram_tensor("disp", [NT, E], F32, kind="ExternalOutput")
    comb_d = nc.dram_tensor("comb", [NT, E], F32, kind="ExternalOutput")
    zstat_d = nc.dram_tensor("zstat", [P, TILES], F32, kind="ExternalOutput")

    with tile.TileContext(nc) as tc:
        with (
            tc.tile_pool(name="const", bufs=1) as const,
            tc.tile_pool(name="xp", bufs=3) as xp,
            tc.tile_pool(name="lgsb", bufs=2) as lgsb,
            tc.tile_pool(name="outp", bufs=2) as outp,
            tc.tile_pool(name="ep", bufs=4) as ep,
            tc.tile_pool(name="psAB", bufs=2, space="PSUM") as psAB,
            tc.tile_pool(name="psL", bufs=4, space="PSUM") as psL,
        ):
            # identS: rows 0:64 hold I_64, rows 64:128 hold I_64 / 2048.
            identS = const.tile([P, E], F32)
            nc.gpsimd.memset(identS[:], 0.0)
            nc.gpsimd.affine_select(
                out=identS[0:E, :], in_=identS[0:E, :],
                compare_op=ALU.not_equal, fill=1.0,
                base=0, pattern=[[-1, E]], channel_multiplier=1,
            )
            nc.gpsimd.affine_select(
                out=identS[E:2 * E, :], in_=identS[E:2 * E, :],
                compare_op=ALU.not_equal, fill=1.0,
                base=0, pattern=[[-1, E]], channel_multiplier=1,
            )

            whl_sb = const.tile([P, CHUNKS, 2 * E], F16)
            nc.sync.dma_start(whl_sb[:], whl_d.rearrange("(c p) e -> p c e", p=P))
            whz_sb = const.tile([P, CHUNKS, 2 * E], F16)
            nc.sync.dma_start(whz_sb[:], whz_d.rearrange("(c p) e -> p c e", p=P))

            zstat = const.tile([P, TILES], F32)

            for g in range(GROUPS):
                ts_ = slice(g * GROUP, (g + 1) * GROUP)
                xh_g = xp.tile([P, CHUNKS, GROUP], F16, name="xh_g")
                nc.sync.dma_start(
                    xh_g[:], xh_d[:, ts_].rearrange("(c p) t -> p c t", p=P)
                )
                xl_g = xp.tile([P, CHUNKS, GROUP], F16, name="xl_g")
                nc.sync.dma_start(
                    xl_g[:], xl_d[:, ts_].rearrange("(c p) t -> p c t", p=P)
                )

                # All matmuls are uniform full-width [128,128] stationary so
                # the PE's background weight buffer keeps LDW/MM pipelined.
                # whz = [0 | wh]: the zero half adds 0 into rows 0:64.
                AB = psAB.tile([P, GROUP], F32)
                for c in range(CHUNKS):
                    nc.tensor.matmul(
                        AB[:], whl_sb[:, c, :], xh_g[:, c, :],
                        start=(c == 0), stop=False,
                    )
                for c in range(CHUNKS):
                    nc.tensor.matmul(
                        AB[:], whz_sb[:, c, :], xl_g[:, c, :],
                        start=False, stop=(c == CHUNKS - 1),
                    )

                AB_sb = lgsb.tile([P, GROUP], F32)
                nc.vector.tensor_copy(AB_sb[0:E, :], AB[0:E, :])
                nc.vector.tensor_scalar(
                    AB_sb[E:2 * E, :], AB[E:2 * E, :], 1.0 / LO_SCALE, None,
                    op0=ALU.mult,
                )

                gdt = outp.tile([P, TILES_PER_GROUP, E], F32, name="gdt")
                gct = outp.tile([P, TILES_PER_GROUP, E], F32, name="gct")

                for j in range(TILES_PER_GROUP):
                    t_idx = g * TILES_PER_GROUP + j
                    jsl = slice(j * P, (j + 1) * P)
                    lg_ps = psL.tile([P, E], F32)
                    nc.tensor.matmul(
                        lg_ps[:], AB_sb[0:E, jsl], identS[0:E, :],
                        is_transpose=True, start=True, stop=False,
                    )
                    nc.tensor.matmul(
                        lg_ps[:], AB_sb[E:2 * E, jsl], identS[E:2 * E, :],
                        is_transpose=True, start=False, stop=True,
                    )
                    lg = ep.tile([P, E], F32)
                    nc.vector.tensor_copy(lg[:], lg_ps[:])

                    mx = ep.tile([P, 8], F32)
                    nc.vector.max(mx[:], lg[:])
                    negm = ep.tile([P, 1], F32)
                    nc.vector.tensor_scalar_mul(negm[:], mx[:, 0:1], -1.0)

                    et = ep.tile([P, E], F32)
                    ssum = ep.tile([P, 1], F32)
                    nc.scalar.activation(
                        et[:], lg[:], AF.Exp, bias=negm[:], scale=1.0,
                        accum_out=ssum[:],
                    )
                    rec = ep.tile([P, 1], F32)
                    nc.vector.reciprocal(rec[:], ssum[:])
                    probs = ep.tile([P, E], F32)
                    nc.vector.tensor_scalar_mul(probs[:], et[:], rec[:])

                    dt = gdt[:, j, :]
                    nc.vector.tensor_scalar(
                        dt, lg[:], mx[:, 1:2], None, op0=ALU.is_ge
                    )
                    ct = gct[:, j, :]
                    nc.vector.tensor_tensor(ct, probs[:], dt, op=ALU.mult)

                    # z-loss partial: zstat[:, t] = sum_e exp(probs)
                    ee = ep.tile([P, E], F32)
                    nc.scalar.activation(
                        ee[:], probs[:], AF.Exp,
                        accum_out=zstat[:, t_idx:t_idx + 1],
                    )

                nc.scalar.dma_start(
                    disp_d[ts_, :].rearrange("(j p) e -> p j e", p=P), gdt[:]
                )
                nc.scalar.dma_start(
                    comb_d[ts_, :].rearrange("(j p) e -> p j e", p=P), gct[:]
                )

            nc.scalar.dma_start(zstat_d[:], zstat[:])

    nc.compile()
    return nc


def _get_module():
    global _nc
    if _nc is None:
        _nc = _build_module()
    return _nc


def _split16(a):
    hi = a.astype(np.float16)
    lo = ((a - hi.astype(np.float32)) * np.float32(LO_SCALE)).astype(np.float16)
    return hi, lo


def _make_in_maps(hidden_states, gate_weight):
    x = np.asarray(hidden_states, dtype=np.float32).reshape(N, D)
    w = np.asarray(gate_weight, dtype=np.float32)
    wt = np.ascontiguousarray(w.T)  # [D, E]
    wh, wl = _split16(wt)
    whl = np.concatenate([wh, wl], axis=1)               # [D, 128]
    whz = np.concatenate([np.zeros_like(wh), wh], axis=1)  # [D, 128]
    in_maps = []
    for i in range(N_CORES):
        xT = np.ascontiguousarray(x[i * NT:(i + 1) * NT].T)  # [D, NT]
        xh, xl = _split16(xT)
        in_maps.append({"xh": xh, "xl": xl, "whl": whl, "whz": whz})
    return in_maps


def _postprocess(results):
    disp0 = np.concatenate([results[i]["disp"] for i in range(N_CORES)], axis=0)
    comb0 = np.concatenate([results[i]["comb"] for i in range(N_CORES)], axis=0)
    zstat = np.stack([results[i]["zstat"] for i in range(N_CORES)])  # [8,128,16]

    dsum = disp0.sum(axis=0, dtype=np.float64)   # [E]
    csum = comb0.sum(axis=0, dtype=np.float64)   # [E]
    lse = np.log(zstat.astype(np.float64))
    z_loss = np.float32((lse ** 2).mean())

    gates_mean = csum / N
    selection_mean = dsum / N
    lb_loss = np.float32((gates_mean * selection_mean).sum() * E)

    dispatch = np.zeros((N, E, TOPK), np.float32)
    dispatch[:, :, 0] = disp0
    combine = np.zeros((N, E, TOPK), np.float32)
    combine[:, :, 0] = comb0
    return (
        dispatch.reshape(B, S, E, TOPK),
        combine.reshape(B, S, E, TOPK),
        lb_loss,
        z_loss,
    )


def run_on_device(in_maps, trace=False, **kwargs):
    from concourse.bass_utils import run_bass_kernel_spmd

    nc = _get_module()
    return run_bass_kernel_spmd(
        nc, in_maps, list(range(N_CORES)), trace=trace, **kwargs
    )


def kernel(hidden_states, gate_weight):
    in_maps = _make_in_maps(hidden_states, gate_weight)
    res = run_on_device(in_maps)
    return _postprocess(res.results)
